# revision 14
# baseline (speedup 1.0000x reference)
"""Trainium2 Bass kernel for retention-style causal MHA + out-proj + residual + LayerNorm.

Sharding: 8 cores = 4 batches x 2 query-parities. Core c handles batch c//2 and
query blocks {2i + c%2, i=0..7} (128 rows each).

Design notes (this toolchain/HW):
- PE matmuls with operands at base partition != 0 hard-fault the device, so
  every matmul operand lives at partition 0: per-head (rank) q/k slices are
  scattered into [32, N] tiles via SBUF->SBUF DMA after dense projection.
- start=True clears has_written bits for the WHOLE 2KB psum bank; only the
  first matmul touching a fresh psum tile carries it.  Subsequent first
  touches overwrite (bit unset) and later matmuls accumulate.
- Decay sparsity: exp(-g*d) makes distant-key scores ~0, so exp(score) ~ 1:
  for key blocks far enough below the diagonal the softmax contribution is
  uniform and equals a prefix-sum over V (plus key counts for the
  denominator), applied with one rank-1 (ones) matmul per (qblock, head).
  Per-head cutoffs come from the gammas at first call; heads are processed
  in gamma-ascending "rank" order so active head sets are rank prefixes.
- exp runs only on the scalar engine; everything else is kept off it.
- The block-causal mask is added into score psum via an identity-stationary
  matmul accumulate (keeps the add off the vector engine).
- dtype bf16 for matmul operands, fp32 psum/LN; rel-err gate is 2e-2.
"""

import math
import numpy as np

B, S, D, H, DH = 4, 2048, 256, 8, 32
QB = 8          # query blocks per core
NB = 16         # key blocks per batch
VW = 36         # padded per-head slot in v/av (16B-aligned psum offsets)
NCORES = 8
LN_EPS = 1e-5
TAU = 3e-2      # decay threshold: block exact iff exp(-g*dmin) >= TAU

_CACHE = {}


# ---------------------------------------------------------------- fallback
def _reference_numpy(Q, K, V, mask, gammas, Wq, bq, Wk, bk, Wv, bv, Wo, bo, ln_g, ln_b):
    q = (Q @ Wq + bq).reshape(B, S, H, DH)
    k = (K @ Wk + bk).reshape(B, S, H, DH)
    v = (V @ Wv + bv).reshape(B, S, H, DH)
    scores = np.einsum("bshd,bthd->bhst", q, k) / np.sqrt(DH).astype(np.float32)
    pos = np.arange(S)
    dist = np.abs(pos[:, None] - pos[None, :]).astype(np.float32)
    decay = np.exp(-gammas[:, None, None] * dist[None])
    scores = scores * decay[None]
    scores = np.where(mask[None, None] == 0, np.float32(-10000.0), scores)
    scores = scores - scores.max(-1, keepdims=True)
    e = np.exp(scores)
    attn = e / e.sum(-1, keepdims=True)
    out = np.einsum("bhst,bthd->bshd", attn, v).reshape(B, S, D)
    out = out @ Wo + bo
    x = Q + out
    mu = x.mean(-1, keepdims=True)
    var = ((x - mu) ** 2).mean(-1, keepdims=True)
    return ((x - mu) / np.sqrt(var + LN_EPS) * ln_g + ln_b).astype(np.float32)


# ---------------------------------------------------------------- plan
def _plan_i(i, cuts):
    """Chunks and psum-strip packing for query block index i (g0 = 2i).

    chunk = [kb, r0, nr, slot]: nr ranks starting at r0 of key block kb;
    slot None for plain causal blocks, 0/1 for the two diagonal-band slots
    (which get the additive mask matmul).
    Returns (chunks, strips); strip = {'fill': [f0, f1], 'items': [(ci, bank, off)]}.
    """
    chunks = []
    for kb in range(2 * i):
        dist = 2 * i - kb
        m = sum(1 for c in cuts if c >= dist)
        if m == 0:
            continue
        chunks.append([kb, 0, min(m, 4), None])
        if m > 4:
            chunks.append([kb, 4, m - 4, None])
    for slot, kb in ((0, 2 * i), (1, 2 * i + 1)):
        chunks.append([kb, 0, 4, slot])
        chunks.append([kb, 4, 4, slot])

    strips = []
    for ci in sorted(range(len(chunks)), key=lambda j: -chunks[j][2]):
        w = chunks[ci][2] * 128
        placed = False
        for st in strips:
            for b in (0, 1):
                if st["fill"][b] + w <= 512:
                    st["items"].append((ci, b, st["fill"][b]))
                    st["fill"][b] += w
                    placed = True
                    break
            if placed:
                break
        if not placed:
            strips.append({"fill": [w, 0], "items": [(ci, 0, 0)]})
    return chunks, strips


# ---------------------------------------------------------------- bass build
def _build_nc(cuts):
    import concourse.bacc as bacc
    import concourse.mybir as mybir
    from concourse.tile import TileContext

    f32 = mybir.dt.float32
    bf16 = mybir.dt.bfloat16
    AF = mybir.ActivationFunctionType
    AX = mybir.AxisListType.X

    nc = bacc.Bacc("TRN2", target_bir_lowering=False, debug=False, num_devices=NCORES)

    qs_d = nc.dram_tensor("qs", [QB * 128, D], bf16, kind="ExternalInput")
    xqT_d = nc.dram_tensor("xqT", [D, QB * 128], bf16, kind="ExternalInput")
    xkT_d = nc.dram_tensor("xkT", [D, S], bf16, kind="ExternalInput")
    xvT_d = nc.dram_tensor("xvT", [D, S], bf16, kind="ExternalInput")
    wq_d = nc.dram_tensor("wq", [D, D], bf16, kind="ExternalInput")
    wk_d = nc.dram_tensor("wk", [D, D], bf16, kind="ExternalInput")
    wv_d = nc.dram_tensor("wv", [D, H * VW], bf16, kind="ExternalInput")
    wo_d = nc.dram_tensor("wo", [D, D], bf16, kind="ExternalInput")
    at_d = nc.dram_tensor("at", [D, QB * 128], bf16, kind="ExternalInput")
    bt_d = nc.dram_tensor("bt", [D, S], bf16, kind="ExternalInput")
    amx_d = nc.dram_tensor("amx", [128, 1024], bf16, kind="ExternalInput")
    vmask_d = nc.dram_tensor("vmask", [128, H * VW], bf16, kind="ExternalInput")
    idb_d = nc.dram_tensor("idb", [128, 128], bf16, kind="ExternalInput")
    idf_d = nc.dram_tensor("idf", [128, 128], f32, kind="ExternalInput")
    out_d = nc.dram_tensor("out", [QB * 128, D], f32, kind="ExternalOutput")

    with TileContext(nc) as tc:
        with (
            tc.tile_pool(name="const", bufs=1) as cp,
            tc.tile_pool(name="estrip", bufs=3) as ep,
            tc.tile_pool(name="attn", bufs=2) as ap_,
            tc.tile_pool(name="xwork", bufs=4) as xw,
            tc.tile_pool(name="small", bufs=12) as sm,
            tc.tile_pool(name="spsum", bufs=2, space="PSUM") as sp_p,
            tc.tile_pool(name="avpsum", bufs=2, space="PSUM") as av_p,
            tc.tile_pool(name="wpsum", bufs=2, space="PSUM") as w_p,
        ):
            # ---------------- constants into SBUF
            def load(dram, p0, ncols, tag, dt=bf16):
                t = cp.tile([128, ncols], dt, tag=tag, name=tag)
                nc.sync.dma_start(t[:], dram[p0 * 128:(p0 + 1) * 128, :ncols])
                return t

            wq_sb = [load(wq_d, k, D, f"wq{k}") for k in range(2)]
            wk_sb = [load(wk_d, k, D, f"wk{k}") for k in range(2)]
            wv_sb = [load(wv_d, k, H * VW, f"wv{k}") for k in range(2)]
            wo_sb = [load(wo_d, k, D, f"wo{k}") for k in range(2)]
            at_sb = [load(at_d, m, QB * 128, f"at{m}") for m in range(2)]
            bt_sb = [load(bt_d, m, S, f"bt{m}") for m in range(2)]
            xqT_sb = [load(xqT_d, m, QB * 128, f"xqT{m}") for m in range(2)]
            xkT_sb = [load(xkT_d, m, S, f"xkT{m}") for m in range(2)]
            xvT_sb = [load(xvT_d, m, S, f"xvT{m}") for m in range(2)]
            amx_sb = load(amx_d, 0, 1024, "amx")
            vmask_sb = load(vmask_d, 0, H * VW, "vmask")
            idb_sb = load(idb_d, 0, 128, "idb")
            idf_sb = load(idf_d, 0, 128, "idf", dt=f32)
            ones_sb = cp.tile([128, 128], f32, tag="ones", name="ones")
            nc.gpsimd.memset(ones_sb[:], 1.0)
            eps_sb = cp.tile([128, 1], f32, tag="eps", name="eps")
            nc.gpsimd.memset(eps_sb[:], LN_EPS)

            qs_sb = []
            for t in range(QB):
                q = cp.tile([128, D], bf16, tag=f"qs{t}", name=f"qs{t}")
                nc.sync.dma_start(q[:], qs_d[t * 128:(t + 1) * 128, :])
                qs_sb.append(q)

            # ---------------- dense projections (feature-major, rank-permuted)
            qTd = [cp.tile([128, QB * 128], bf16, tag=f"qTd{m}", name=f"qTd{m}") for m in range(2)]
            kTd = [cp.tile([128, S], bf16, tag=f"kTd{m}", name=f"kTd{m}") for m in range(2)]

            def proj_T(dst, w_sb, xT, sc_sb, width):
                for m in range(2):
                    for n0 in range(0, width, 512):
                        ps = w_p.tile([128, 512], f32, tag="work", name="work")
                        for k in range(2):
                            nc.tensor.matmul(
                                ps[:, :512],
                                lhsT=w_sb[k][:, m * 128:(m + 1) * 128],
                                rhs=xT[k][:, n0:n0 + 512],
                                start=(k == 0), stop=(k == 1),
                            )
                        nc.vector.tensor_mul(
                            dst[m][:, n0:n0 + 512], ps[:, :512], sc_sb[m][:, n0:n0 + 512]
                        )

            proj_T(qTd, wq_sb, xqT_sb, at_sb, QB * 128)
            proj_T(kTd, wk_sb, xkT_sb, bt_sb, S)

            # scatter rank slices to base-partition-0 tiles (SBUF->SBUF DMA)
            qTr, kTr = [], []
            for r in range(H):
                m, rr = r // 4, 32 * (r % 4)
                qt = cp.tile([32, QB * 128], bf16, tag=f"qTr{r}", name=f"qTr{r}")
                nc.sync.dma_start(qt[:], qTd[m][rr:rr + 32, :])
                qTr.append(qt)
                kt = cp.tile([32, S], bf16, tag=f"kTr{r}", name=f"kTr{r}")
                nc.sync.dma_start(kt[:], kTd[m][rr:rr + 32, :])
                kTr.append(kt)

            # ---------------- v projection (+ ones column via vmask add)
            v_sb = []
            for t in range(NB):
                ps = w_p.tile([128, 512], f32, tag="work", name="work")
                for k in range(2):
                    nc.tensor.matmul(
                        ps[:, :H * VW],
                        lhsT=xvT_sb[k][:, t * 128:(t + 1) * 128],
                        rhs=wv_sb[k][:, :H * VW],
                        start=(k == 0), stop=(k == 1),
                    )
                v = cp.tile([128, H * VW], bf16, tag=f"v{t}", name=f"v{t}")
                nc.vector.tensor_add(v[:], ps[:, :H * VW], vmask_sb[:])
                v_sb.append(v)

            # prefix sums of v blocks (uniform far-key contributions), fp32
            pv = []
            p0 = cp.tile([128, H * VW], f32, tag="pv0", name="pv0")
            nc.gpsimd.memset(p0[:], 0.0)
            pv.append(p0)
            for j in range(1, NB):
                pj = cp.tile([128, H * VW], f32, tag=f"pv{j}", name=f"pv{j}")
                nc.gpsimd.tensor_add(pj[:], pv[j - 1][:], v_sb[j - 1][:])
                pv.append(pj)

            # ---------------- attention per query block
            attnT = [cp.tile([128, QB * 128], bf16, tag=f"attnT{m}", name=f"attnT{m}") for m in range(2)]
            x8 = [cp.tile([128, D], f32, tag=f"x8_{t}", name=f"x8_{t}") for t in range(QB)]
            mu8 = cp.tile([128, QB], f32, tag="mu8", name="mu8")
            var8 = cp.tile([128, QB], f32, tag="var8", name="var8")

            for i in range(QB):
                chunks, strips = _plan_i(i, cuts)
                av = av_p.tile([128, H * VW], f32, tag="av", name="av")
                # uniform far-prefix contributions; r==0 carries the single
                # bank-clearing start for this tile's accumulation.
                for r in range(H):
                    j = max(0, 2 * i - cuts[r])
                    nc.tensor.matmul(
                        av[:, r * VW:(r + 1) * VW],
                        lhsT=ones_sb[:, :128],
                        rhs=pv[j][:, r * VW:(r + 1) * VW],
                        start=(r == 0), stop=False,
                        skip_group_check=True,
                    )

                n_av = sum(c[2] for c in chunks)
                done_av = 0
                for st in strips:
                    sp = sp_p.tile([128, 1024], f32, tag="scores", name="scores")
                    bank_started = [False, False]
                    for (ci, b, off) in st["items"]:
                        kb, r0, nr, slot = chunks[ci]
                        for rr in range(nr):
                            r = r0 + rr
                            co = b * 512 + off + rr * 128
                            nc.tensor.matmul(
                                sp[:, co:co + 128],
                                lhsT=kTr[r][:, kb * 128:(kb + 1) * 128],
                                rhs=qTr[r][:, i * 128:(i + 1) * 128],
                                start=(not bank_started[b]), stop=False,
                                skip_group_check=True,
                            )
                            bank_started[b] = True
                        if slot is not None:
                            # the [128,128] pattern repeats across ranks, so
                            # any nr*128-wide window of the slot's 512 works
                            co = b * 512 + off
                            nc.tensor.matmul(
                                sp[:, co:co + nr * 128],
                                lhsT=idb_sb[:, :128],
                                rhs=amx_sb[:, slot * 512:slot * 512 + nr * 128],
                                start=False, stop=True,
                                skip_group_check=True,
                            )
                    es = ep.tile([128, 1024], bf16, tag="estrip", name="estrip")
                    for b in (0, 1):
                        if st["fill"][b]:
                            nc.scalar.activation(
                                es[:, b * 512:b * 512 + st["fill"][b]],
                                sp[:, b * 512:b * 512 + st["fill"][b]],
                                AF.Exp,
                            )
                    for (ci, b, off) in st["items"]:
                        kb, r0, nr, slot = chunks[ci]
                        for rr in range(nr):
                            r = r0 + rr
                            co = b * 512 + off + rr * 128
                            done_av += 1
                            nc.tensor.matmul(
                                av[:, r * VW:(r + 1) * VW],
                                lhsT=es[:, co:co + 128],
                                rhs=v_sb[kb][:, r * VW:(r + 1) * VW],
                                start=False, stop=(done_av == n_av),
                                skip_group_check=True,
                            )

                # normalize: attn[:, r*32:+32] = av_num / av_den per rank
                av_s = xw.tile([128, H * VW], f32, tag="avs", name="avs")
                nc.vector.tensor_copy(av_s[:], av[:])
                den8 = sm.tile([128, H], f32, tag="den8", name="den8")
                for r in range(H):
                    nc.gpsimd.tensor_copy(den8[:, r:r + 1], av_s[:, r * VW + 32:r * VW + 33])
                rc8 = sm.tile([128, H], f32, tag="rc8", name="rc8")
                nc.vector.reciprocal(rc8[:], den8[:])
                attn = ap_.tile([128, D], f32, tag="attn", name="attn")
                for r in range(H):
                    nc.gpsimd.tensor_scalar_mul(
                        attn[:, r * 32:(r + 1) * 32], av_s[:, r * VW:r * VW + 32], rc8[:, r:r + 1]
                    )

                # transpose attn for the out-projection (bf16 in -> f32 psum)
                for m in range(2):
                    tp = w_p.tile([128, 512], f32, tag="work", name="work")
                    nc.tensor.transpose(tp[:, :128], attn[:, m * 128:(m + 1) * 128], idf_sb[:])
                    nc.vector.tensor_copy(attnT[m][:, i * 128:(i + 1) * 128], tp[:, :128])

                # out-proj + residual + LN stats (sqrt deferred to tail)
                po = w_p.tile([128, 512], f32, tag="work", name="work")
                for k in range(2):
                    nc.tensor.matmul(
                        po[:, :D],
                        lhsT=attnT[k][:, i * 128:(i + 1) * 128],
                        rhs=wo_sb[k][:, :D],
                        start=(k == 0), stop=(k == 1),
                    )
                x = x8[i]
                nc.vector.tensor_add(x[:], po[:, :D], qs_sb[i][:])
                su = sm.tile([128, 1], f32, tag="su", name="su")
                nc.vector.reduce_sum(su[:], x[:], axis=AX)
                nc.gpsimd.tensor_scalar_mul(mu8[:, i:i + 1], su[:], 1.0 / D)
                xc = xw.tile([128, D], f32, tag="xc", name="xc")
                nc.gpsimd.tensor_scalar_sub(xc[:], x[:], mu8[:, i:i + 1])
                sq = xw.tile([128, D], f32, tag="sq", name="sq")
                nc.gpsimd.tensor_mul(sq[:], xc[:], xc[:])
                sv = sm.tile([128, 1], f32, tag="sv", name="sv")
                nc.vector.reduce_sum(sv[:], sq[:], axis=AX)
                nc.gpsimd.tensor_scalar_mul(var8[:, i:i + 1], sv[:], 1.0 / D)

            # ---------------- LN tail (batched so the act table loads once)
            for i in range(QB):
                sd = sm.tile([128, 1], f32, tag="sd", name="sd")
                nc.scalar.activation(sd[:], var8[:, i:i + 1], AF.Sqrt, bias=eps_sb[:])
                rs = sm.tile([128, 1], f32, tag="rs", name="rs")
                nc.vector.reciprocal(rs[:], sd[:])
                y = xw.tile([128, D], f32, tag="y", name="y")
                nc.vector.tensor_scalar(
                    y[:], x8[i][:], mu8[:, i:i + 1], rs[:],
                    mybir.AluOpType.subtract, mybir.AluOpType.mult,
                )
                nc.sync.dma_start(out_d[i * 128:(i + 1) * 128, :], y[:])

    nc.finalize()
    return nc


# ---------------------------------------------------------------- entry
def kernel(Q, K, V, mask, gammas, Wq, bq, Wk, bk, Wv, bv, Wo, bo, ln_g, ln_b):
    import ml_dtypes

    bf = ml_dtypes.bfloat16
    args = [np.asarray(a) for a in (Q, K, V, mask, gammas, Wq, bq, Wk, bk, Wv, bv, Wo, bo, ln_g, ln_b)]
    Q, K, V, mask, gammas, Wq, bq, Wk, bk, Wv, bv, Wo, bo, ln_g, ln_b = args

    tril = np.tril(np.ones((S, S), mask.dtype))
    fast = (
        np.array_equal(mask, tril)
        and not np.any(bq) and not np.any(bk) and not np.any(bv) and not np.any(bo)
        and not np.any(ln_b) and np.all(ln_g == 1.0)
        and np.all(gammas > 0) and float(np.max(gammas)) * (S - 1) < 80.0
    )
    if not fast:
        return _reference_numpy(*args)

    from concourse.bass_utils import run_bass_kernel_spmd

    order = np.argsort(gammas.astype(np.float64), kind="stable")  # rank -> head
    g_r = gammas.astype(np.float64)[order]
    L = math.log(1.0 / TAU)
    cuts = tuple(
        int(min(NB, max(1, math.floor(1.0 + (L / g - 1.0) / 128.0)))) for g in g_r
    )

    key = ("nc", cuts)
    if key not in _CACHE:
        _CACHE[key] = _build_nc(cuts)
    nc = _CACHE[key]

    perm = np.concatenate([np.arange(o * 32, o * 32 + 32) for o in order])
    sc = float(DH) ** -0.25
    pos = np.arange(S, dtype=np.float64)

    wq_p = Wq[:, perm].astype(bf)
    wk_p = Wk[:, perm].astype(bf)
    wo_p = Wo[perm, :].astype(bf)
    wv_ext = np.zeros((D, H * VW), np.float32)
    vmask = np.zeros((128, H * VW), np.float32)
    for r in range(H):
        o = order[r]
        wv_ext[:, r * VW:r * VW + 32] = Wv[:, o * 32:(o + 1) * 32]
        vmask[:, r * VW + 32] = 1.0
    wv_ext = wv_ext.astype(bf)
    vmask = vmask.astype(bf)

    bt = np.empty((D, S), np.float64)
    for r in range(H):
        bt[r * 32:(r + 1) * 32, :] = np.exp(g_r[r] * pos[None, :]) * sc
    bt = bt.astype(bf)

    # block-diagonal-band additive masks (same pattern for every i)
    kloc = np.arange(128)[:, None]
    qloc = np.arange(128)[None, :]
    tri = np.where(kloc <= qloc, 0.0, -10000.0).astype(np.float32)  # [key, query]
    zeros = np.zeros((128, 128), np.float32)
    full = np.full((128, 128), -10000.0, np.float32)

    ident = np.eye(128, dtype=np.float32).astype(bf)

    in_maps = []
    for c in range(NCORES):
        b, p = c // 2, c % 2
        rows = np.concatenate([np.arange((2 * i + p) * 128, (2 * i + p + 1) * 128) for i in range(QB)])
        at = np.empty((D, QB * 128), np.float64)
        for r in range(H):
            at[r * 32:(r + 1) * 32, :] = np.exp(-g_r[r] * rows[None, :].astype(np.float64)) * sc
        slotA = tri if p == 0 else zeros
        slotB = full if p == 0 else tri
        amx = np.concatenate([np.tile(slotA, (1, 4)), np.tile(slotB, (1, 4))], axis=1)
        in_maps.append({
            "qs": np.ascontiguousarray(Q[b][rows]).astype(bf),
            "xqT": np.ascontiguousarray(Q[b][rows].T).astype(bf),
            "xkT": np.ascontiguousarray(K[b].T).astype(bf),
            "xvT": np.ascontiguousarray(V[b].T).astype(bf),
            "wq": wq_p, "wk": wk_p, "wv": wv_ext, "wo": wo_p,
            "at": at.astype(bf), "bt": bt,
            "amx": amx.astype(bf), "vmask": vmask, "idb": ident,
            "idf": np.eye(128, dtype=np.float32),
        })

    res = run_bass_kernel_spmd(nc, in_maps, list(range(NCORES)))
    _CACHE["last_results"] = res

    out = np.empty((B, S, D), np.float32)
    for c in range(NCORES):
        b, p = c // 2, c % 2
        o = res.results[c]["out"]
        for i in range(QB):
            g = 2 * i + p
            out[b, g * 128:(g + 1) * 128, :] = o[i * 128:(i + 1) * 128, :]
    return out


# revision 15
# speedup vs baseline: 1.6112x; 1.6112x over previous
"""Trainium2 Bass kernel for retention-style causal MHA + out-proj + residual + LayerNorm.

Sharding: 8 cores = 4 batches x 2 query-parities. Core c handles batch c//2 and
query blocks {2i + c%2, i=0..7} (128 rows each).

Design notes (this toolchain/HW):
- PE matmuls with operands at base partition != 0 hard-fault the device, so
  every matmul operand lives at partition 0: per-head (rank) q/k slices are
  scattered into [32, N] tiles via SBUF->SBUF DMA after dense projection.
- start=True clears has_written bits for the WHOLE 2KB psum bank; only the
  first matmul touching a fresh psum tile carries it.  Subsequent first
  touches overwrite (bit unset) and later matmuls accumulate.
- Decay sparsity: exp(-g*d) makes distant-key scores ~0, so exp(score) ~ 1:
  for key blocks far enough below the diagonal the softmax contribution is
  uniform and equals a prefix-sum over V (plus key counts for the
  denominator), applied with one rank-1 (ones) matmul per (qblock, head).
  Per-head cutoffs come from the gammas at first call; heads are processed
  in gamma-ascending "rank" order so active head sets are rank prefixes.
- exp runs only on the scalar engine; everything else is kept off it.
- The block-causal mask is added into score psum via an identity-stationary
  matmul accumulate (keeps the add off the vector engine).
- dtype bf16 for matmul operands, fp32 psum/LN; rel-err gate is 2e-2.
"""

import math
import numpy as np

B, S, D, H, DH = 4, 2048, 256, 8, 32
QB = 8          # query blocks per core
NB = 16         # key blocks per batch
VW = 36         # padded per-head slot in v/av (16B-aligned psum offsets)
NCORES = 8
LN_EPS = 1e-5
TAU = 3e-2      # decay threshold: block exact iff exp(-g*dmin) >= TAU

_CACHE = {}


# ---------------------------------------------------------------- fallback
def _reference_numpy(Q, K, V, mask, gammas, Wq, bq, Wk, bk, Wv, bv, Wo, bo, ln_g, ln_b):
    q = (Q @ Wq + bq).reshape(B, S, H, DH)
    k = (K @ Wk + bk).reshape(B, S, H, DH)
    v = (V @ Wv + bv).reshape(B, S, H, DH)
    scores = np.einsum("bshd,bthd->bhst", q, k) / np.sqrt(DH).astype(np.float32)
    pos = np.arange(S)
    dist = np.abs(pos[:, None] - pos[None, :]).astype(np.float32)
    decay = np.exp(-gammas[:, None, None] * dist[None])
    scores = scores * decay[None]
    scores = np.where(mask[None, None] == 0, np.float32(-10000.0), scores)
    scores = scores - scores.max(-1, keepdims=True)
    e = np.exp(scores)
    attn = e / e.sum(-1, keepdims=True)
    out = np.einsum("bhst,bthd->bshd", attn, v).reshape(B, S, D)
    out = out @ Wo + bo
    x = Q + out
    mu = x.mean(-1, keepdims=True)
    var = ((x - mu) ** 2).mean(-1, keepdims=True)
    return ((x - mu) / np.sqrt(var + LN_EPS) * ln_g + ln_b).astype(np.float32)


# ---------------------------------------------------------------- plan
def _plan_i(i, cuts):
    """Chunks and psum-strip packing for query block index i (g0 = 2i).

    chunk = [kb, r0, nr, slot]: nr ranks starting at r0 of key block kb;
    slot None for plain causal blocks, 0/1 for the two diagonal-band slots
    (which get the additive mask matmul).
    Returns (chunks, strips); strip = {'fill': [f0, f1], 'items': [(ci, bank, off)]}.
    """
    chunks = []
    for kb in range(2 * i):
        dist = 2 * i - kb
        m = sum(1 for c in cuts if c >= dist)
        if m == 0:
            continue
        chunks.append([kb, 0, min(m, 4), None])
        if m > 4:
            chunks.append([kb, 4, m - 4, None])
    for slot, kb in ((0, 2 * i), (1, 2 * i + 1)):
        chunks.append([kb, 0, 4, slot])
        chunks.append([kb, 4, 4, slot])

    strips = []
    for ci in sorted(range(len(chunks)), key=lambda j: -chunks[j][2]):
        w = chunks[ci][2] * 128
        placed = False
        for st in strips:
            for b in (0, 1):
                if st["fill"][b] + w <= 512:
                    st["items"].append((ci, b, st["fill"][b]))
                    st["fill"][b] += w
                    placed = True
                    break
            if placed:
                break
        if not placed:
            strips.append({"fill": [w, 0], "items": [(ci, 0, 0)]})
    return chunks, strips


# ---------------------------------------------------------------- bass build
def _build_nc(cuts):
    import concourse.bacc as bacc
    import concourse.mybir as mybir
    from concourse.tile import TileContext

    f32 = mybir.dt.float32
    bf16 = mybir.dt.bfloat16
    AF = mybir.ActivationFunctionType
    AX = mybir.AxisListType.X

    nc = bacc.Bacc("TRN2", target_bir_lowering=False, debug=False, num_devices=NCORES)

    qs_d = nc.dram_tensor("qs", [QB * 128, D], bf16, kind="ExternalInput")
    xqT_d = nc.dram_tensor("xqT", [D, QB * 128], bf16, kind="ExternalInput")
    xkT_d = nc.dram_tensor("xkT", [D, S], bf16, kind="ExternalInput")
    xvT_d = nc.dram_tensor("xvT", [D, S], bf16, kind="ExternalInput")
    wq_d = nc.dram_tensor("wq", [D, D], bf16, kind="ExternalInput")
    wk_d = nc.dram_tensor("wk", [D, D], bf16, kind="ExternalInput")
    wv_d = nc.dram_tensor("wv", [D, H * VW], bf16, kind="ExternalInput")
    wo_d = nc.dram_tensor("wo", [D, D], bf16, kind="ExternalInput")
    at_d = nc.dram_tensor("at", [D, QB * 128], bf16, kind="ExternalInput")
    bt_d = nc.dram_tensor("bt", [D, S], bf16, kind="ExternalInput")
    amx_d = nc.dram_tensor("amx", [128, 1024], bf16, kind="ExternalInput")
    vmask_d = nc.dram_tensor("vmask", [128, H * VW], bf16, kind="ExternalInput")
    idb_d = nc.dram_tensor("idb", [128, 128], bf16, kind="ExternalInput")
    idf_d = nc.dram_tensor("idf", [128, 128], f32, kind="ExternalInput")
    out_d = nc.dram_tensor("out", [QB * 128, D], f32, kind="ExternalOutput")

    with TileContext(nc) as tc:
        with (
            tc.tile_pool(name="const", bufs=1) as cp,
            tc.tile_pool(name="estrip", bufs=3) as ep,
            tc.tile_pool(name="attn", bufs=2) as ap_,
            tc.tile_pool(name="xwork", bufs=4) as xw,
            tc.tile_pool(name="small", bufs=12) as sm,
            tc.tile_pool(name="spsum", bufs=2, space="PSUM") as sp_p,
            tc.tile_pool(name="avpsum", bufs=2, space="PSUM") as av_p,
            tc.tile_pool(name="wpsum", bufs=2, space="PSUM") as w_p,
        ):
            # ---------------- constants into SBUF
            def load(dram, p0, ncols, tag, dt=bf16):
                t = cp.tile([128, ncols], dt, tag=tag, name=tag)
                nc.sync.dma_start(t[:], dram[p0 * 128:(p0 + 1) * 128, :ncols])
                return t

            wq_sb = [load(wq_d, k, D, f"wq{k}") for k in range(2)]
            wk_sb = [load(wk_d, k, D, f"wk{k}") for k in range(2)]
            wv_sb = [load(wv_d, k, H * VW, f"wv{k}") for k in range(2)]
            wo_sb = [load(wo_d, k, D, f"wo{k}") for k in range(2)]
            at_sb = [load(at_d, m, QB * 128, f"at{m}") for m in range(2)]
            bt_sb = [load(bt_d, m, S, f"bt{m}") for m in range(2)]
            xqT_sb = [load(xqT_d, m, QB * 128, f"xqT{m}") for m in range(2)]
            xkT_sb = [load(xkT_d, m, S, f"xkT{m}") for m in range(2)]
            xvT_sb = [load(xvT_d, m, S, f"xvT{m}") for m in range(2)]
            amx_sb = load(amx_d, 0, 1024, "amx")
            vmask_sb = load(vmask_d, 0, H * VW, "vmask")
            idb_sb = load(idb_d, 0, 128, "idb")
            idf_sb = load(idf_d, 0, 128, "idf", dt=f32)
            ones_sb = cp.tile([128, 128], f32, tag="ones", name="ones")
            nc.gpsimd.memset(ones_sb[:], 1.0)
            eps_sb = cp.tile([128, 1], f32, tag="eps", name="eps")
            nc.gpsimd.memset(eps_sb[:], LN_EPS)

            qs_sb = []
            for t in range(QB):
                q = cp.tile([128, D], bf16, tag=f"qs{t}", name=f"qs{t}")
                nc.sync.dma_start(q[:], qs_d[t * 128:(t + 1) * 128, :])
                qs_sb.append(q)

            # ---------------- dense projections (feature-major, rank-permuted)
            qTd = [cp.tile([128, QB * 128], bf16, tag=f"qTd{m}", name=f"qTd{m}") for m in range(2)]
            kTd = [cp.tile([128, S], bf16, tag=f"kTd{m}", name=f"kTd{m}") for m in range(2)]

            def proj_T(dst, w_sb, xT, sc_sb, width):
                for m in range(2):
                    for n0 in range(0, width, 512):
                        ps = w_p.tile([128, 512], f32, tag="work", name="work")
                        for k in range(2):
                            nc.tensor.matmul(
                                ps[:, :512],
                                lhsT=w_sb[k][:, m * 128:(m + 1) * 128],
                                rhs=xT[k][:, n0:n0 + 512],
                                start=(k == 0), stop=(k == 1),
                            )
                        nc.vector.tensor_mul(
                            dst[m][:, n0:n0 + 512], ps[:, :512], sc_sb[m][:, n0:n0 + 512]
                        )

            proj_T(qTd, wq_sb, xqT_sb, at_sb, QB * 128)
            proj_T(kTd, wk_sb, xkT_sb, bt_sb, S)

            # block-diagonal q: rank slot j of group g at rows 32j, cols
            # i*512 + j*128; zeros elsewhere make a single full-K matmul per
            # chunk compute per-rank scores against dense kTd.
            qbd = []
            for g in range(2):
                qb = cp.tile([128, QB * 512], bf16, tag=f"qbd{g}", name=f"qbd{g}")
                nc.vector.memset(qb[:], 0.0)
                qbd.append(qb)
            for g in range(2):
                for j in range(4):
                    src = qTd[g][32 * j:32 * j + 32, :].rearrange(
                        "p (i c) -> p i c", i=QB, c=128)
                    dst = qbd[g][32 * j:32 * j + 32, :].rearrange(
                        "p (i c) -> p i c", i=QB, c=512)[:, :, j * 128:(j + 1) * 128]
                    nc.sync.dma_start(dst, src)

            # ---------------- v projection (+ ones column via vmask add)
            v_sb = []
            for t in range(NB):
                ps = w_p.tile([128, 512], f32, tag="work", name="work")
                for k in range(2):
                    nc.tensor.matmul(
                        ps[:, :H * VW],
                        lhsT=xvT_sb[k][:, t * 128:(t + 1) * 128],
                        rhs=wv_sb[k][:, :H * VW],
                        start=(k == 0), stop=(k == 1),
                    )
                v = cp.tile([128, H * VW], bf16, tag=f"v{t}", name=f"v{t}")
                nc.vector.tensor_add(v[:], ps[:, :H * VW], vmask_sb[:])
                v_sb.append(v)

            # prefix sums of v blocks (uniform far-key contributions), fp32
            pv = []
            p0 = cp.tile([128, H * VW], f32, tag="pv0", name="pv0")
            nc.gpsimd.memset(p0[:], 0.0)
            pv.append(p0)
            for j in range(1, NB):
                pj = cp.tile([128, H * VW], f32, tag=f"pv{j}", name=f"pv{j}")
                nc.gpsimd.tensor_add(pj[:], pv[j - 1][:], v_sb[j - 1][:])
                pv.append(pj)

            # ---------------- attention per query block
            attnT = [cp.tile([128, QB * 128], bf16, tag=f"attnT{m}", name=f"attnT{m}") for m in range(2)]
            x8 = [cp.tile([128, D], f32, tag=f"x8_{t}", name=f"x8_{t}") for t in range(QB)]
            mu8 = cp.tile([128, QB], f32, tag="mu8", name="mu8")
            var8 = cp.tile([128, QB], f32, tag="var8", name="var8")

            for i in range(QB):
                chunks, strips = _plan_i(i, cuts)
                av = av_p.tile([128, H * VW], f32, tag="av", name="av")
                # uniform far-prefix contributions; first MM carries the
                # single bank-clearing start for this tile's accumulation.
                runs = []
                for r in range(H):
                    j = max(0, 2 * i - cuts[r])
                    if runs and runs[-1][0] == j:
                        runs[-1][2] += 1
                    else:
                        runs.append([j, r, 1])
                for ri, (j, r0, nr) in enumerate(runs):
                    nc.tensor.matmul(
                        av[:, r0 * VW:(r0 + nr) * VW],
                        lhsT=ones_sb[:, :128],
                        rhs=pv[j][:, r0 * VW:(r0 + nr) * VW],
                        start=(ri == 0), stop=False,
                        skip_group_check=True,
                    )

                n_av = sum(c[2] for c in chunks)
                done_av = 0
                for st in strips:
                    sp = sp_p.tile([128, 1024], f32, tag="scores", name="scores")
                    bank_started = [False, False]
                    for (ci, b, off) in st["items"]:
                        kb, r0, nr, slot = chunks[ci]
                        g = r0 // 4
                        co = b * 512 + off
                        nc.tensor.matmul(
                            sp[:, co:co + nr * 128],
                            lhsT=kTd[g][:, kb * 128:(kb + 1) * 128],
                            rhs=qbd[g][:, i * 512:i * 512 + nr * 128],
                            start=(not bank_started[b]), stop=False,
                            skip_group_check=True,
                        )
                        bank_started[b] = True
                        if slot is not None:
                            # the [128,128] pattern repeats across ranks, so
                            # any nr*128-wide window of the slot's 512 works
                            co = b * 512 + off
                            nc.tensor.matmul(
                                sp[:, co:co + nr * 128],
                                lhsT=idb_sb[:, :128],
                                rhs=amx_sb[:, slot * 512:slot * 512 + nr * 128],
                                start=False, stop=True,
                                skip_group_check=True,
                            )
                    es = ep.tile([128, 1024], bf16, tag="estrip", name="estrip")
                    for b in (0, 1):
                        if st["fill"][b]:
                            nc.scalar.activation(
                                es[:, b * 512:b * 512 + st["fill"][b]],
                                sp[:, b * 512:b * 512 + st["fill"][b]],
                                AF.Exp,
                            )
                    for (ci, b, off) in st["items"]:
                        kb, r0, nr, slot = chunks[ci]
                        for rr in range(nr):
                            r = r0 + rr
                            co = b * 512 + off + rr * 128
                            done_av += 1
                            nc.tensor.matmul(
                                av[:, r * VW:(r + 1) * VW],
                                lhsT=es[:, co:co + 128],
                                rhs=v_sb[kb][:, r * VW:(r + 1) * VW],
                                start=False, stop=(done_av == n_av),
                                skip_group_check=True,
                            )

                # normalize: attn = av_num * (1/av_den), two DVE ops via
                # strided/broadcast access patterns
                rc8 = sm.tile([128, H], f32, tag="rc8", name="rc8")
                nc.vector.reciprocal(rc8[:], av[:, 32:H * VW:VW])
                attn = ap_.tile([128, D], f32, tag="attn", name="attn")
                num_v = av[:].rearrange("p (h w) -> p h w", h=H, w=VW)[:, :, 0:32]
                rc_v = rc8[:].unsqueeze(2).broadcast_to([128, H, 32])
                attn_v = attn[:].rearrange("p (h w) -> p h w", h=H, w=32)
                nc.vector.tensor_mul(attn_v, num_v, rc_v)

                # transpose attn for the out-projection (bf16 in -> f32 psum)
                for m in range(2):
                    tp = w_p.tile([128, 512], f32, tag="work", name="work")
                    nc.tensor.transpose(tp[:, :128], attn[:, m * 128:(m + 1) * 128], idf_sb[:])
                    nc.vector.tensor_copy(attnT[m][:, i * 128:(i + 1) * 128], tp[:, :128])

                # out-proj + residual + LN stats (sqrt deferred to tail)
                po = w_p.tile([128, 512], f32, tag="work", name="work")
                for k in range(2):
                    nc.tensor.matmul(
                        po[:, :D],
                        lhsT=attnT[k][:, i * 128:(i + 1) * 128],
                        rhs=wo_sb[k][:, :D],
                        start=(k == 0), stop=(k == 1),
                    )
                x = x8[i]
                nc.vector.tensor_add(x[:], po[:, :D], qs_sb[i][:])
                su = sm.tile([128, 1], f32, tag="su", name="su")
                nc.vector.reduce_sum(su[:], x[:], axis=AX)
                nc.gpsimd.tensor_scalar_mul(mu8[:, i:i + 1], su[:], 1.0 / D)
                xc = xw.tile([128, D], f32, tag="xc", name="xc")
                nc.gpsimd.tensor_scalar_sub(xc[:], x[:], mu8[:, i:i + 1])
                sq = xw.tile([128, D], f32, tag="sq", name="sq")
                nc.gpsimd.tensor_mul(sq[:], xc[:], xc[:])
                sv = sm.tile([128, 1], f32, tag="sv", name="sv")
                nc.vector.reduce_sum(sv[:], sq[:], axis=AX)
                nc.gpsimd.tensor_scalar_mul(var8[:, i:i + 1], sv[:], 1.0 / D)

            # ---------------- LN tail: rs = exp(-0.5*ln(var+eps)); Ln and
            # Exp live in the same activation table set as the attention
            # exps, so no table reloads.
            for i in range(QB):
                lnv = sm.tile([128, 1], f32, tag="lnv", name="lnv")
                nc.scalar.activation(lnv[:], var8[:, i:i + 1], AF.Ln, bias=eps_sb[:])
                rs = sm.tile([128, 1], f32, tag="rs", name="rs")
                nc.scalar.activation(rs[:], lnv[:], AF.Exp, scale=-0.5)
                y = xw.tile([128, D], f32, tag="y", name="y")
                nc.vector.tensor_scalar(
                    y[:], x8[i][:], mu8[:, i:i + 1], rs[:],
                    mybir.AluOpType.subtract, mybir.AluOpType.mult,
                )
                nc.sync.dma_start(out_d[i * 128:(i + 1) * 128, :], y[:])

    nc.finalize()
    return nc


# ---------------------------------------------------------------- entry
def kernel(Q, K, V, mask, gammas, Wq, bq, Wk, bk, Wv, bv, Wo, bo, ln_g, ln_b):
    import ml_dtypes

    bf = ml_dtypes.bfloat16
    args = [np.asarray(a) for a in (Q, K, V, mask, gammas, Wq, bq, Wk, bk, Wv, bv, Wo, bo, ln_g, ln_b)]
    Q, K, V, mask, gammas, Wq, bq, Wk, bk, Wv, bv, Wo, bo, ln_g, ln_b = args

    tril = np.tril(np.ones((S, S), mask.dtype))
    fast = (
        np.array_equal(mask, tril)
        and not np.any(bq) and not np.any(bk) and not np.any(bv) and not np.any(bo)
        and not np.any(ln_b) and np.all(ln_g == 1.0)
        and np.all(gammas > 0) and float(np.max(gammas)) * (S - 1) < 80.0
    )
    if not fast:
        return _reference_numpy(*args)

    from concourse.bass_utils import run_bass_kernel_spmd

    order = np.argsort(gammas.astype(np.float64), kind="stable")  # rank -> head
    g_r = gammas.astype(np.float64)[order]
    L = math.log(1.0 / TAU)
    cuts = tuple(
        int(min(NB, max(1, math.floor(1.0 + (L / g - 1.0) / 128.0)))) for g in g_r
    )

    key = ("nc", cuts)
    if key not in _CACHE:
        _CACHE[key] = _build_nc(cuts)
    nc = _CACHE[key]

    perm = np.concatenate([np.arange(o * 32, o * 32 + 32) for o in order])
    sc = float(DH) ** -0.25
    pos = np.arange(S, dtype=np.float64)

    wq_p = Wq[:, perm].astype(bf)
    wk_p = Wk[:, perm].astype(bf)
    wo_p = Wo[perm, :].astype(bf)
    wv_ext = np.zeros((D, H * VW), np.float32)
    vmask = np.zeros((128, H * VW), np.float32)
    for r in range(H):
        o = order[r]
        wv_ext[:, r * VW:r * VW + 32] = Wv[:, o * 32:(o + 1) * 32]
        vmask[:, r * VW + 32] = 1.0
    wv_ext = wv_ext.astype(bf)
    vmask = vmask.astype(bf)

    bt = np.empty((D, S), np.float64)
    for r in range(H):
        bt[r * 32:(r + 1) * 32, :] = np.exp(g_r[r] * pos[None, :]) * sc
    bt = bt.astype(bf)

    # block-diagonal-band additive masks (same pattern for every i)
    kloc = np.arange(128)[:, None]
    qloc = np.arange(128)[None, :]
    tri = np.where(kloc <= qloc, 0.0, -10000.0).astype(np.float32)  # [key, query]
    zeros = np.zeros((128, 128), np.float32)
    full = np.full((128, 128), -10000.0, np.float32)

    ident = np.eye(128, dtype=np.float32).astype(bf)

    in_maps = []
    for c in range(NCORES):
        b, p = c // 2, c % 2
        rows = np.concatenate([np.arange((2 * i + p) * 128, (2 * i + p + 1) * 128) for i in range(QB)])
        at = np.empty((D, QB * 128), np.float64)
        for r in range(H):
            at[r * 32:(r + 1) * 32, :] = np.exp(-g_r[r] * rows[None, :].astype(np.float64)) * sc
        slotA = tri if p == 0 else zeros
        slotB = full if p == 0 else tri
        amx = np.concatenate([np.tile(slotA, (1, 4)), np.tile(slotB, (1, 4))], axis=1)
        in_maps.append({
            "qs": np.ascontiguousarray(Q[b][rows]).astype(bf),
            "xqT": np.ascontiguousarray(Q[b][rows].T).astype(bf),
            "xkT": np.ascontiguousarray(K[b].T).astype(bf),
            "xvT": np.ascontiguousarray(V[b].T).astype(bf),
            "wq": wq_p, "wk": wk_p, "wv": wv_ext, "wo": wo_p,
            "at": at.astype(bf), "bt": bt,
            "amx": amx.astype(bf), "vmask": vmask, "idb": ident,
            "idf": np.eye(128, dtype=np.float32),
        })

    res = run_bass_kernel_spmd(nc, in_maps, list(range(NCORES)))
    _CACHE["last_results"] = res

    out = np.empty((B, S, D), np.float32)
    for c in range(NCORES):
        b, p = c // 2, c % 2
        o = res.results[c]["out"]
        for i in range(QB):
            g = 2 * i + p
            out[b, g * 128:(g + 1) * 128, :] = o[i * 128:(i + 1) * 128, :]
    return out


# revision 19
# speedup vs baseline: 2.1762x; 1.3507x over previous
"""Trainium2 Bass kernel for retention-style causal MHA + out-proj + residual + LayerNorm.

Sharding: 8 cores = 4 batches x 2 query-parities. Core c handles batch c//2 and
query blocks {2i + c%2, i=0..7} (128 rows each).

Design notes (this toolchain/HW):
- PE matmuls with operands at base partition != 0 hard-fault the device, so
  every matmul operand lives at partition 0: per-head (rank) q/k slices are
  scattered into [32, N] tiles via SBUF->SBUF DMA after dense projection.
- start=True clears has_written bits for the WHOLE 2KB psum bank; only the
  first matmul touching a fresh psum tile carries it.  Subsequent first
  touches overwrite (bit unset) and later matmuls accumulate.
- Decay sparsity: exp(-g*d) makes distant-key scores ~0, so exp(score) ~ 1:
  for key blocks far enough below the diagonal the softmax contribution is
  uniform and equals a prefix-sum over V (plus key counts for the
  denominator), applied with one rank-1 (ones) matmul per (qblock, head).
  Per-head cutoffs come from the gammas at first call; heads are processed
  in gamma-ascending "rank" order so active head sets are rank prefixes.
- exp runs only on the scalar engine; everything else is kept off it.
- The block-causal mask is added into score psum via an identity-stationary
  matmul accumulate (keeps the add off the vector engine).
- dtype bf16 for matmul operands, fp32 psum/LN; rel-err gate is 2e-2.
"""

import math
import numpy as np

B, S, D, H, DH = 4, 2048, 256, 8, 32
QB = 8          # query blocks per core
NB = 16         # key blocks per batch
VW = 36         # padded per-head slot in v/av (16B-aligned psum offsets)
NCORES = 8
LN_EPS = 1e-5
TAU = 3e-2      # decay threshold: block exact iff exp(-g*dmin) >= TAU

_CACHE = {}


# ---------------------------------------------------------------- fallback
def _reference_numpy(Q, K, V, mask, gammas, Wq, bq, Wk, bk, Wv, bv, Wo, bo, ln_g, ln_b):
    q = (Q @ Wq + bq).reshape(B, S, H, DH)
    k = (K @ Wk + bk).reshape(B, S, H, DH)
    v = (V @ Wv + bv).reshape(B, S, H, DH)
    scores = np.einsum("bshd,bthd->bhst", q, k) / np.sqrt(DH).astype(np.float32)
    pos = np.arange(S)
    dist = np.abs(pos[:, None] - pos[None, :]).astype(np.float32)
    decay = np.exp(-gammas[:, None, None] * dist[None])
    scores = scores * decay[None]
    scores = np.where(mask[None, None] == 0, np.float32(-10000.0), scores)
    scores = scores - scores.max(-1, keepdims=True)
    e = np.exp(scores)
    attn = e / e.sum(-1, keepdims=True)
    out = np.einsum("bhst,bthd->bshd", attn, v).reshape(B, S, D)
    out = out @ Wo + bo
    x = Q + out
    mu = x.mean(-1, keepdims=True)
    var = ((x - mu) ** 2).mean(-1, keepdims=True)
    return ((x - mu) / np.sqrt(var + LN_EPS) * ln_g + ln_b).astype(np.float32)


# ---------------------------------------------------------------- plan
def _plan_i(i, cuts):
    """Chunks and psum-strip packing for query block index i (g0 = 2i).

    chunk = [kb, r0, nr, slot]: nr ranks starting at r0 of key block kb;
    slot None for plain causal blocks, 0/1 for the two diagonal-band slots
    (which get the additive mask matmul).
    Returns (chunks, strips); strip = {'fill': [f0, f1], 'items': [(ci, bank, off)]}.
    """
    chunks = []
    for kb in range(2 * i):
        dist = 2 * i - kb
        m = sum(1 for c in cuts if c >= dist)
        if m == 0:
            continue
        chunks.append([kb, 0, min(m, 4), None])
        if m > 4:
            chunks.append([kb, 4, m - 4, None])
    for slot, kb in ((0, 2 * i), (1, 2 * i + 1)):
        chunks.append([kb, 0, 4, slot])
        chunks.append([kb, 4, 4, slot])

    strips = []
    for ci in sorted(range(len(chunks)), key=lambda j: -chunks[j][2]):
        w = chunks[ci][2] * 128
        placed = False
        for st in strips:
            for b in (0, 1):
                if st["fill"][b] + w <= 512:
                    st["items"].append((ci, b, st["fill"][b]))
                    st["fill"][b] += w
                    placed = True
                    break
            if placed:
                break
        if not placed:
            strips.append({"fill": [w, 0], "items": [(ci, 0, 0)]})
    return chunks, strips


# ---------------------------------------------------------------- bass build
def _build_nc(cuts):
    import concourse.bacc as bacc
    import concourse.mybir as mybir
    from concourse.tile import TileContext

    f32 = mybir.dt.float32
    bf16 = mybir.dt.bfloat16
    AF = mybir.ActivationFunctionType
    AX = mybir.AxisListType.X

    nc = bacc.Bacc("TRN2", target_bir_lowering=False, debug=False, num_devices=NCORES)

    qs_d = nc.dram_tensor("qs", [QB * 128, D], bf16, kind="ExternalInput")
    xqT_d = nc.dram_tensor("xqT", [D, QB * 128], bf16, kind="ExternalInput")
    xkT_d = nc.dram_tensor("xkT", [D, S], bf16, kind="ExternalInput")
    xvT_d = nc.dram_tensor("xvT", [D, S], bf16, kind="ExternalInput")
    wq_d = nc.dram_tensor("wq", [D, D], bf16, kind="ExternalInput")
    wk_d = nc.dram_tensor("wk", [D, D], bf16, kind="ExternalInput")
    wv_d = nc.dram_tensor("wv", [D, H * VW], bf16, kind="ExternalInput")
    wo_d = nc.dram_tensor("wo", [D, D], bf16, kind="ExternalInput")
    at_d = nc.dram_tensor("at", [D, QB * 128], bf16, kind="ExternalInput")
    bt_d = nc.dram_tensor("bt", [D, S], bf16, kind="ExternalInput")
    amx_d = nc.dram_tensor("amx", [128, 1024], bf16, kind="ExternalInput")
    vmask_d = nc.dram_tensor("vmask", [128, H * VW], bf16, kind="ExternalInput")
    idb_d = nc.dram_tensor("idb", [128, 128], bf16, kind="ExternalInput")
    idf_d = nc.dram_tensor("idf", [128, 128], f32, kind="ExternalInput")
    out_d = nc.dram_tensor("out", [QB * 128, D], f32, kind="ExternalOutput")

    with TileContext(nc) as tc:
        with (
            tc.tile_pool(name="const", bufs=1) as cp,
            tc.tile_pool(name="estrip", bufs=3) as ep,
            tc.tile_pool(name="attn", bufs=2) as ap_,
            tc.tile_pool(name="xwork", bufs=4) as xw,
            tc.tile_pool(name="small", bufs=12) as sm,
            tc.tile_pool(name="spsum", bufs=2, space="PSUM") as sp_p,
            tc.tile_pool(name="avpsum", bufs=2, space="PSUM") as av_p,
            tc.tile_pool(name="wpsum", bufs=2, space="PSUM") as w_p,
        ):
            # ---------------- constants into SBUF
            def load(dram, p0, ncols, tag, dt=bf16):
                t = cp.tile([128, ncols], dt, tag=tag, name=tag)
                nc.sync.dma_start(t[:], dram[p0 * 128:(p0 + 1) * 128, :ncols])
                return t

            wq_sb = [load(wq_d, k, D, f"wq{k}") for k in range(2)]
            wk_sb = [load(wk_d, k, D, f"wk{k}") for k in range(2)]
            wv_sb = [load(wv_d, k, H * VW, f"wv{k}") for k in range(2)]
            wo_sb = [load(wo_d, k, D, f"wo{k}") for k in range(2)]
            at_sb = [load(at_d, m, QB * 128, f"at{m}") for m in range(2)]
            bt_sb = [load(bt_d, m, S, f"bt{m}") for m in range(2)]
            xqT_sb = [load(xqT_d, m, QB * 128, f"xqT{m}") for m in range(2)]
            xkT_sb = [load(xkT_d, m, S, f"xkT{m}") for m in range(2)]
            xvT_sb = [load(xvT_d, m, S, f"xvT{m}") for m in range(2)]
            amx_sb = load(amx_d, 0, 1024, "amx")
            vmask_sb = load(vmask_d, 0, H * VW, "vmask")
            idb_sb = load(idb_d, 0, 128, "idb")
            idf_sb = load(idf_d, 0, 128, "idf", dt=f32)
            ones_sb = cp.tile([128, 128], f32, tag="ones", name="ones")
            nc.gpsimd.memset(ones_sb[:], 1.0)
            eps_sb = cp.tile([128, 1], f32, tag="eps", name="eps")
            nc.gpsimd.memset(eps_sb[:], LN_EPS)

            qs_sb = []
            for t in range(QB):
                q = cp.tile([128, D], bf16, tag=f"qs{t}", name=f"qs{t}")
                nc.sync.dma_start(q[:], qs_d[t * 128:(t + 1) * 128, :])
                qs_sb.append(q)

            # ---------------- dense projections (feature-major, rank-permuted)
            qTd = [cp.tile([128, QB * 128], bf16, tag=f"qTd{m}", name=f"qTd{m}") for m in range(2)]
            kTd = [cp.tile([128, S], bf16, tag=f"kTd{m}", name=f"kTd{m}") for m in range(2)]

            def proj_T(dst, w_sb, xT, sc_sb, width):
                for m in range(2):
                    for n0 in range(0, width, 512):
                        ps = w_p.tile([128, 512], f32, tag="work", name="work")
                        for k in range(2):
                            nc.tensor.matmul(
                                ps[:, :512],
                                lhsT=w_sb[k][:, m * 128:(m + 1) * 128],
                                rhs=xT[k][:, n0:n0 + 512],
                                start=(k == 0), stop=(k == 1),
                            )
                        nc.vector.tensor_mul(
                            dst[m][:, n0:n0 + 512], ps[:, :512], sc_sb[m][:, n0:n0 + 512]
                        )

            proj_T(qTd, wq_sb, xqT_sb, at_sb, QB * 128)
            proj_T(kTd, wk_sb, xkT_sb, bt_sb, S)

            # block-diagonal q: rank slot j of group g at rows 32j, cols
            # i*512 + j*128; zeros elsewhere make a single full-K matmul per
            # chunk compute per-rank scores against dense kTd.
            qbd = []
            for g in range(2):
                qb = cp.tile([128, QB * 512], bf16, tag=f"qbd{g}", name=f"qbd{g}")
                nc.vector.memset(qb[:], 0.0)
                qbd.append(qb)
            for g in range(2):
                for j in range(4):
                    src = qTd[g][32 * j:32 * j + 32, :].rearrange(
                        "p (i c) -> p i c", i=QB, c=128)
                    dst = qbd[g][32 * j:32 * j + 32, :].rearrange(
                        "p (i c) -> p i c", i=QB, c=512)[:, :, j * 128:(j + 1) * 128]
                    nc.sync.dma_start(dst, src)

            # ---------------- v projection (+ ones column via vmask add)
            v_sb = []
            for t in range(NB):
                ps = w_p.tile([128, 512], f32, tag="work", name="work")
                for k in range(2):
                    nc.tensor.matmul(
                        ps[:, :H * VW],
                        lhsT=xvT_sb[k][:, t * 128:(t + 1) * 128],
                        rhs=wv_sb[k][:, :H * VW],
                        start=(k == 0), stop=(k == 1),
                    )
                v = cp.tile([128, H * VW], bf16, tag=f"v{t}", name=f"v{t}")
                nc.vector.tensor_add(v[:], ps[:, :H * VW], vmask_sb[:])
                v_sb.append(v)

            # prefix sums of v blocks (uniform far-key contributions), fp32
            pv = []
            p0 = cp.tile([128, H * VW], f32, tag="pv0", name="pv0")
            nc.gpsimd.memset(p0[:], 0.0)
            pv.append(p0)
            for j in range(1, NB):
                pj = cp.tile([128, H * VW], f32, tag=f"pv{j}", name=f"pv{j}")
                nc.gpsimd.tensor_add(pj[:], pv[j - 1][:], v_sb[j - 1][:])
                pv.append(pj)

            # ---------------- attention per query block
            attnT = [cp.tile([128, QB * 128], bf16, tag=f"attnT{m}", name=f"attnT{m}") for m in range(2)]
            x8 = [cp.tile([128, D], f32, tag=f"x8_{t}", name=f"x8_{t}") for t in range(QB)]
            mu8 = cp.tile([128, QB], f32, tag="mu8", name="mu8")
            var8 = cp.tile([128, QB], f32, tag="var8", name="var8")

            for i in range(QB):
                chunks, strips = _plan_i(i, cuts)
                av = av_p.tile([128, H * VW], f32, tag="av", name="av")
                # uniform far-prefix contributions; first MM carries the
                # single bank-clearing start for this tile's accumulation.
                runs = []
                for r in range(H):
                    j = max(0, 2 * i - cuts[r])
                    if runs and runs[-1][0] == j:
                        runs[-1][2] += 1
                    else:
                        runs.append([j, r, 1])
                for ri, (j, r0, nr) in enumerate(runs):
                    nc.tensor.matmul(
                        av[:, r0 * VW:(r0 + nr) * VW],
                        lhsT=ones_sb[:, :128],
                        rhs=pv[j][:, r0 * VW:(r0 + nr) * VW],
                        start=(ri == 0), stop=False,
                        skip_group_check=True,
                    )

                n_av = sum(c[2] for c in chunks)
                done_av = 0
                for st in strips:
                    sp = sp_p.tile([128, 1024], f32, tag="scores", name="scores")
                    bank_started = [False, False]
                    for (ci, b, off) in st["items"]:
                        kb, r0, nr, slot = chunks[ci]
                        g = r0 // 4
                        co = b * 512 + off
                        nc.tensor.matmul(
                            sp[:, co:co + nr * 128],
                            lhsT=kTd[g][:, kb * 128:(kb + 1) * 128],
                            rhs=qbd[g][:, i * 512:i * 512 + nr * 128],
                            start=(not bank_started[b]), stop=False,
                            skip_group_check=True,
                        )
                        bank_started[b] = True
                        if slot is not None:
                            # the [128,128] pattern repeats across ranks, so
                            # any nr*128-wide window of the slot's 512 works
                            co = b * 512 + off
                            nc.tensor.matmul(
                                sp[:, co:co + nr * 128],
                                lhsT=idb_sb[:, :128],
                                rhs=amx_sb[:, slot * 512:slot * 512 + nr * 128],
                                start=False, stop=True,
                                skip_group_check=True,
                            )
                    es = ep.tile([128, 1024], bf16, tag="estrip", name="estrip")
                    for b in (0, 1):
                        if st["fill"][b]:
                            nc.scalar.activation(
                                es[:, b * 512:b * 512 + st["fill"][b]],
                                sp[:, b * 512:b * 512 + st["fill"][b]],
                                AF.Exp,
                            )
                    for (ci, b, off) in st["items"]:
                        kb, r0, nr, slot = chunks[ci]
                        for rr in range(nr):
                            r = r0 + rr
                            co = b * 512 + off + rr * 128
                            done_av += 1
                            nc.tensor.matmul(
                                av[:, r * VW:(r + 1) * VW],
                                lhsT=es[:, co:co + 128],
                                rhs=v_sb[kb][:, r * VW:(r + 1) * VW],
                                start=False, stop=(done_av == n_av),
                                skip_group_check=True,
                            )

                # normalize: attn = av_num * (1/av_den), two DVE ops via
                # strided/broadcast access patterns
                rc8 = sm.tile([128, H], f32, tag="rc8", name="rc8")
                nc.vector.reciprocal(rc8[:], av[:, 32:H * VW:VW])
                attn = ap_.tile([128, D], f32, tag="attn", name="attn")
                num_v = av[:].rearrange("p (h w) -> p h w", h=H, w=VW)[:, :, 0:32]
                rc_v = rc8[:].unsqueeze(2).broadcast_to([128, H, 32])
                attn_v = attn[:].rearrange("p (h w) -> p h w", h=H, w=32)
                nc.vector.tensor_mul(attn_v, num_v, rc_v)

                # transpose attn for the out-projection (bf16 in -> f32 psum)
                for m in range(2):
                    tp = w_p.tile([128, 512], f32, tag="work", name="work")
                    nc.tensor.transpose(tp[:, :128], attn[:, m * 128:(m + 1) * 128], idf_sb[:])
                    nc.vector.tensor_copy(attnT[m][:, i * 128:(i + 1) * 128], tp[:, :128])

                # out-proj + residual + LN stats (sqrt deferred to tail)
                po = w_p.tile([128, 512], f32, tag="work", name="work")
                for k in range(2):
                    nc.tensor.matmul(
                        po[:, :D],
                        lhsT=attnT[k][:, i * 128:(i + 1) * 128],
                        rhs=wo_sb[k][:, :D],
                        start=(k == 0), stop=(k == 1),
                    )
                x = x8[i]
                nc.vector.tensor_add(x[:], po[:, :D], qs_sb[i][:])
                su = sm.tile([128, 1], f32, tag="su", name="su")
                nc.vector.reduce_sum(su[:], x[:], axis=AX)
                nc.vector.tensor_scalar_mul(mu8[:, i:i + 1], su[:], 1.0 / D)
                xc = xw.tile([128, D], f32, tag="xc", name="xc")
                nc.vector.tensor_scalar_sub(xc[:], x[:], mu8[:, i:i + 1])
                sq = xw.tile([128, D], f32, tag="sq", name="sq")
                nc.vector.tensor_mul(sq[:], xc[:], xc[:])
                sv = sm.tile([128, 1], f32, tag="sv", name="sv")
                nc.vector.reduce_sum(sv[:], sq[:], axis=AX)
                nc.vector.tensor_scalar_mul(var8[:, i:i + 1], sv[:], 1.0 / D)

            # ---------------- LN tail: rs = exp(-0.5*ln(var+eps)); Ln and
            # Exp live in the same activation table set as the attention
            # exps, so no table reloads.
            for i in range(QB):
                lnv = sm.tile([128, 1], f32, tag="lnv", name="lnv")
                nc.scalar.activation(lnv[:], var8[:, i:i + 1], AF.Ln, bias=eps_sb[:])
                rs = sm.tile([128, 1], f32, tag="rs", name="rs")
                nc.scalar.activation(rs[:], lnv[:], AF.Exp, scale=-0.5)
                y = xw.tile([128, D], f32, tag="y", name="y")
                nc.vector.tensor_scalar(
                    y[:], x8[i][:], mu8[:, i:i + 1], rs[:],
                    mybir.AluOpType.subtract, mybir.AluOpType.mult,
                )
                nc.sync.dma_start(out_d[i * 128:(i + 1) * 128, :], y[:])

    nc.finalize()
    import os
    if not os.environ.get("NO_ACT_COLLAPSE"):
        _collapse_act_table_loads(nc)
    return nc


def _collapse_act_table_loads(nc):
    """All activation funcs used here (Exp, Ln) live in the
    natural_log_exp_and_others set; keep one load of that set and drop the
    rest so the scalar engine never reloads tables mid-kernel."""
    import concourse.mybir as mybir
    from concourse.hw_specs import get_activation_tables

    tabs = list(get_activation_tables(nc.m.arch).keys())
    set_id = tabs.index("natural_log_exp_and_others")
    first = True
    for func in nc.m.functions:
        for bb in func.blocks:
            keep = []
            for inst in bb.instructions:
                if isinstance(inst, mybir.InstLoadActFuncSet):
                    si = inst.sync_info
                    has_sync = si is not None and (si.on_wait or si.on_update)
                    if first:
                        inst.act_func_set_id = set_id
                        first = False
                        keep.append(inst)
                    elif has_sync:
                        # keep for its sync effects, but reloading the same
                        # set is cheap-ish; just retarget it
                        inst.act_func_set_id = set_id
                        keep.append(inst)
                else:
                    keep.append(inst)
            bb.instructions = keep


# ---------------------------------------------------------------- entry
def kernel(Q, K, V, mask, gammas, Wq, bq, Wk, bk, Wv, bv, Wo, bo, ln_g, ln_b):
    import ml_dtypes

    bf = ml_dtypes.bfloat16
    args = [np.asarray(a) for a in (Q, K, V, mask, gammas, Wq, bq, Wk, bk, Wv, bv, Wo, bo, ln_g, ln_b)]
    Q, K, V, mask, gammas, Wq, bq, Wk, bk, Wv, bv, Wo, bo, ln_g, ln_b = args

    tril = np.tril(np.ones((S, S), mask.dtype))
    fast = (
        np.array_equal(mask, tril)
        and not np.any(bq) and not np.any(bk) and not np.any(bv) and not np.any(bo)
        and not np.any(ln_b) and np.all(ln_g == 1.0)
        and np.all(gammas > 0) and float(np.max(gammas)) * (S - 1) < 80.0
    )
    if not fast:
        return _reference_numpy(*args)

    from concourse.bass_utils import run_bass_kernel_spmd

    order = np.argsort(gammas.astype(np.float64), kind="stable")  # rank -> head
    g_r = gammas.astype(np.float64)[order]
    L = math.log(1.0 / TAU)
    cuts = tuple(
        int(min(NB, max(1, math.floor(1.0 + (L / g - 1.0) / 128.0)))) for g in g_r
    )

    key = ("nc", cuts)
    if key not in _CACHE:
        _CACHE[key] = _build_nc(cuts)
    nc = _CACHE[key]

    perm = np.concatenate([np.arange(o * 32, o * 32 + 32) for o in order])
    sc = float(DH) ** -0.25
    pos = np.arange(S, dtype=np.float64)

    wq_p = Wq[:, perm].astype(bf)
    wk_p = Wk[:, perm].astype(bf)
    wo_p = Wo[perm, :].astype(bf)
    wv_ext = np.zeros((D, H * VW), np.float32)
    vmask = np.zeros((128, H * VW), np.float32)
    for r in range(H):
        o = order[r]
        wv_ext[:, r * VW:r * VW + 32] = Wv[:, o * 32:(o + 1) * 32]
        vmask[:, r * VW + 32] = 1.0
    wv_ext = wv_ext.astype(bf)
    vmask = vmask.astype(bf)

    bt = np.empty((D, S), np.float64)
    for r in range(H):
        bt[r * 32:(r + 1) * 32, :] = np.exp(g_r[r] * pos[None, :]) * sc
    bt = bt.astype(bf)

    # block-diagonal-band additive masks (same pattern for every i)
    kloc = np.arange(128)[:, None]
    qloc = np.arange(128)[None, :]
    tri = np.where(kloc <= qloc, 0.0, -10000.0).astype(np.float32)  # [key, query]
    zeros = np.zeros((128, 128), np.float32)
    full = np.full((128, 128), -10000.0, np.float32)

    ident = np.eye(128, dtype=np.float32).astype(bf)

    in_maps = []
    for c in range(NCORES):
        b, p = c // 2, c % 2
        rows = np.concatenate([np.arange((2 * i + p) * 128, (2 * i + p + 1) * 128) for i in range(QB)])
        at = np.empty((D, QB * 128), np.float64)
        for r in range(H):
            at[r * 32:(r + 1) * 32, :] = np.exp(-g_r[r] * rows[None, :].astype(np.float64)) * sc
        slotA = tri if p == 0 else zeros
        slotB = full if p == 0 else tri
        amx = np.concatenate([np.tile(slotA, (1, 4)), np.tile(slotB, (1, 4))], axis=1)
        in_maps.append({
            "qs": np.ascontiguousarray(Q[b][rows]).astype(bf),
            "xqT": np.ascontiguousarray(Q[b][rows].T).astype(bf),
            "xkT": np.ascontiguousarray(K[b].T).astype(bf),
            "xvT": np.ascontiguousarray(V[b].T).astype(bf),
            "wq": wq_p, "wk": wk_p, "wv": wv_ext, "wo": wo_p,
            "at": at.astype(bf), "bt": bt,
            "amx": amx.astype(bf), "vmask": vmask, "idb": ident,
            "idf": np.eye(128, dtype=np.float32),
        })

    res = run_bass_kernel_spmd(nc, in_maps, list(range(NCORES)))
    _CACHE["last_results"] = res

    out = np.empty((B, S, D), np.float32)
    for c in range(NCORES):
        b, p = c // 2, c % 2
        o = res.results[c]["out"]
        for i in range(QB):
            g = 2 * i + p
            out[b, g * 128:(g + 1) * 128, :] = o[i * 128:(i + 1) * 128, :]
    return out
